# revision 14
# baseline (speedup 1.0000x reference)
"""Causal multi-head self-attention on 8 TRN2 NeuronCores.

Sharding: batch (2) x head-group (4 heads = 256 contiguous features) -> 8 cores.
Each core computes q/k/v projections for its 256 output features from its
batch's full activations, then causal attention for its 4 heads. No
collectives: the host concatenates the 8 shards.

Kernel layout choices (v3, bf16):
  - the host pre-transposes and casts x -> xT [D, S] bf16 and W -> W^T
    [D, 256] bf16, so the device does NO transposes and no PSUM->SBUF
    staging copies for x or W; all matmuls run bf16 (1 cyc/col on the PE,
    vs ~2 cyc/col measured for fp32r at K=64/M=65).
  - qT/kT stored [d, s] (head dim on partitions) so scores come out
    [k, q]; PV consumes exp(scores) directly as the moving operand.
  - score matmuls for a head pair (partitions 0-63 / 64-127) are emitted
    back-to-back: the K=64 stationaries land in disjoint PE row groups
    (tile_position auto-derived from base_partition), so the two matmuls
    execute concurrently in the array -> ~2x on the score stage.
  - softmax skips max-subtraction (scores ~ N(0,1) after the 1/8 scale).
  - causal mask: additive -8e9 on the diagonal 128x128 score blocks in
    PSUM, issued right after the score matmul (hidden behind the previous
    exp, off the exp->PV critical hop); fully-masked column windows are
    never computed.
  - scores for two consecutive key blocks share one 2-bank PSUM tile and
    (off-diagonal) one [128,1024] exp instruction, halving ACT overhead.
  - row sums ride along PV via a ones-column appended to V (65-wide PV
    stationary). Normalization transposes PV output back to [q, hd] on the
    PE (per-partition reciprocals are ~30x cheaper on DVE than per-column
    ones); the PSUM->SBUF staging copy of PV output rides on the idle DMA
    engines instead of DVE.
  - b_v is added during the v_aug staging copy against a host-replicated
    [128, 256] bias tile (no bias matmuls); b_q/b_k ride in the qT/kT
    staging tensor_scalar ops as per-partition scalars.
  - the schedule is emitted as generators explicitly interleaved in
    program order: attention for query group g interleaves with the
    projection units of s-group g+1 (causality makes group g
    data-complete after s-group g).
"""

import sys

import numpy as np

sys.path.insert(0, "/opt/trn_rl_repo")

import ml_dtypes

import concourse.bass as bass
import concourse.tile as tile
from concourse import bacc, mybir
from concourse.bass_utils import run_bass_kernel_spmd

B, S, D, H, DK = 2, 2048, 1024, 16, 64
NCORES = 8
HD = 256  # output features per core (4 heads x 64)
NHC = 4  # heads per core
NST = S // 128  # 16 s-tiles
NCC = D // 128  # 8 contraction chunks
NG = S // 512  # 4 query groups of 512

f32 = mybir.dt.float32
bf16 = mybir.dt.bfloat16
AF = mybir.ActivationFunctionType
PSUM = bass.MemorySpace.PSUM


def _body(nc, tc, x, wq, wk, wv, bq, bk, bvr, tril, amask, out):
    with (
        tc.tile_pool(name="persist", bufs=1) as persist,
        tc.tile_pool(name="u", bufs=8) as u_pool,
        tc.tile_pool(name="zc", bufs=6) as zc_pool,
        tc.tile_pool(name="small", bufs=4) as small,
        tc.tile_pool(name="psum_sp", bufs=1, space=PSUM) as psum_sp,
        tc.tile_pool(name="psum_zp", bufs=1, space=PSUM) as psum_zp,
        tc.tile_pool(name="psum_pp", bufs=2, space=PSUM) as psum_pp,
    ):
        # ---- constants / small inputs ----
        tril_sb = persist.tile([128, 128], bf16)
        nc.sync.dma_start(out=tril_sb[:], in_=tril)
        amask_sb = persist.tile([128, 128], bf16)
        nc.sync.dma_start(out=amask_sb[:], in_=amask)
        bq_sb = persist.tile([128, 2], f32)
        bk_sb = persist.tile([128, 2], f32)
        bvr_sb = persist.tile([128, HD], bf16)
        nc.sync.dma_start(out=bq_sb[:], in_=bq)
        nc.sync.dma_start(out=bk_sb[:], in_=bk)
        nc.sync.dma_start(out=bvr_sb[:], in_=bvr)

        # preload the exp activation table while the x DMAs run
        warm = small.tile([1, 4], f32, tag="warm", name="warm")
        nc.scalar.activation(warm[:], tril_sb[0:1, 0:4], AF.Exp)

        # ---- weights (pre-transposed on host): [128, cc, hd] ----
        wq_sb = persist.tile([128, NCC, HD], bf16)
        wk_sb = persist.tile([128, NCC, HD], bf16)
        wv_sb = persist.tile([128, NCC, HD], bf16)
        for w_ext, w_sb in ((wq, wq_sb), (wk, wk_sb), (wv, wv_sb)):
            nc.sync.dma_start(
                out=w_sb[:], in_=w_ext.rearrange("(c p) d -> p c d", p=128)
            )

        # ---- x (pre-transposed on host): per-group tiles [128, cc, 512] ----
        xg = []
        for g in range(NG):
            xt = persist.tile([128, NCC, 512], bf16, name=f"xg{g}")
            nc.sync.dma_start(
                out=xt[:],
                in_=x.rearrange("(c p) s -> p c s", p=128)[
                    :, :, bass.ts(g, 512)
                ],
            )
            xg.append(xt)

        qT = persist.tile([128, 2, S], bf16)
        kT = persist.tile([128, 2, S], bf16)
        v_aug = persist.tile([128, NST, NHC, 65], bf16)

        # ones column of v_aug
        nc.vector.memset(v_aug[:, :, :, 64], 1.0)

        # ---- projections for s-group sg (512 queries = 4 s-tiles) ----
        # split into q-only and k/v parts: attention group g needs qT(g) for
        # all its score matmuls but kT(g)/v(g) only at the diagonal blocks,
        # so the k/v projections of group g overlap attention of group g
        def proj_one(w_sb, bias, dstT, sg):
            xt = xg[sg]
            pa = psum_pp.tile([128, 512], f32, tag="pp", name="pa")
            pb = psum_pp.tile([128, 512], f32, tag="pp", name="pb")
            for cc in range(NCC):
                for hdc, pp in ((0, pa), (1, pb)):
                    nc.tensor.matmul(
                        pp[:],
                        lhsT=w_sb[:, cc, bass.ts(hdc, 128)],
                        rhs=xt[:, cc, :],
                        start=(cc == 0),
                        stop=(cc == NCC - 1),
                    )
            for hdc, pp in ((0, pa), (1, pb)):
                nc.vector.tensor_scalar_add(
                    dstT[:, hdc, bass.ts(sg, 512)],
                    pp[:],
                    bias[:, hdc : hdc + 1],
                )

        def gen_proj_q(sg):
            proj_one(wq_sb, bq_sb, qT, sg)
            yield

        def gen_proj_kv(sg):
            proj_one(wk_sb, bk_sb, kT, sg)
            yield
            # v projection for the 4 s-tiles (pairs, alternating banks);
            # b_v is added during the staging copy against the replicated
            # bias tile
            xt = xg[sg]
            for spair in range(2):
                pvs = [
                    psum_pp.tile([128, HD], f32, tag="pp", name=f"pv{stl}")
                    for stl in range(2)
                ]
                for cc in range(NCC):
                    for stl in range(2):
                        nc.tensor.matmul(
                            pvs[stl][:],
                            lhsT=xt[:, cc, bass.ts(spair * 2 + stl, 128)],
                            rhs=wv_sb[:, cc, :],
                            start=(cc == 0),
                            stop=(cc == NCC - 1),
                        )
                for stl in range(2):
                    st = sg * 4 + spair * 2 + stl
                    nc.vector.tensor_add(
                        v_aug[:, st, :, 0:64],
                        pvs[stl][:].rearrange("p (h d) -> p h d", h=NHC),
                        bvr_sb[:].rearrange("p (h d) -> p h d", h=NHC),
                    )
                yield

        # ---- attention for query group g (queries [512g, 512g+512)) ----
        def gen_attn(g):
            nkc = 4 * g + 4
            for pair in range(2):
                hdc = pair
                heads = (2 * pair, 2 * pair + 1)
                zp = {}
                for ci, h in enumerate(heads):
                    zp[h] = psum_zp.tile(
                        [65, 512], f32, tag=f"zp{ci}", name=f"zp{h}"
                    )
                prev = []

                def flush_pv(prev):
                    for pkb, h, u in prev:
                        for ci, kc in enumerate((pkb, pkb + 1)):
                            q0 = max(0, 128 * (kc - 4 * g))
                            nc.tensor.matmul(
                                zp[h][:, q0:512],
                                lhsT=v_aug[:, kc, h, :],
                                rhs=u[:, 512 * ci + q0 : 512 * (ci + 1)],
                                start=(kc == 0),
                                stop=(kc == nkc - 1),
                            )

                for kb in range(0, nkc, 2):
                    sp = {}
                    for ci, h in enumerate(heads):
                        sp[h] = psum_sp.tile(
                            [128, 1024], f32, tag=f"sp{ci}", name=f"sp{h}"
                        )
                    # 4 score MMs; head pair adjacent -> concurrent row tiles
                    for ci, kc in enumerate((kb, kb + 1)):
                        j = kc - 4 * g
                        q0 = max(0, 128 * j)
                        for h in heads:
                            po = (h % 2) * 64
                            nc.tensor.matmul(
                                sp[h][:, 512 * ci + q0 : 512 * (ci + 1)],
                                lhsT=kT[po : po + 64, hdc, bass.ts(kc, 128)],
                                rhs=qT[
                                    po : po + 64,
                                    hdc,
                                    bass.ds(g * 512 + q0, 512 - q0),
                                ],
                                start=True,
                                stop=True,
                            )
                        if j >= 0:
                            qd = 512 * ci + q0
                            for h in heads:
                                nc.vector.tensor_add(
                                    sp[h][:, qd : qd + 128],
                                    sp[h][:, qd : qd + 128],
                                    amask_sb[:],
                                )
                    # exp (one [128,1024] instr off-diagonal, windowed on diag)
                    cur = []
                    diag = kb >= 4 * g
                    for h in heads:
                        u = u_pool.tile([128, 1024], bf16, tag="u", name="u")
                        if diag:
                            for ci, kc in enumerate((kb, kb + 1)):
                                q0 = 128 * (kc - 4 * g)
                                nc.scalar.activation(
                                    u[:, 512 * ci + q0 : 512 * (ci + 1)],
                                    sp[h][:, 512 * ci + q0 : 512 * (ci + 1)],
                                    AF.Exp,
                                    scale=0.125,
                                )
                        else:
                            nc.scalar.activation(
                                u[:], sp[h][:], AF.Exp, scale=0.125
                            )
                        cur.append((h, u))
                    # PV for the previous kb's exp tiles (software pipeline)
                    flush_pv(prev)
                    prev = [(kb, h, u) for h, u in cur]
                    yield
                flush_pv(prev)
                # stage unnormalized PV output (incl. the sums row) to SBUF
                # and ship it; the host divides by the sums row during
                # unsharding
                for h in heads:
                    zc = zc_pool.tile([65, 512], bf16, tag="zc", name="zc")
                    nc.vector.tensor_copy(zc[:], zp[h][:])
                    nc.sync.dma_start(
                        out=out[bass.ds(65 * h, 65), bass.ts(g, 512)],
                        in_=zc[:],
                    )
                    yield

        def drain(gen):
            for _ in gen:
                pass

        def chain(*gens):
            for gen in gens:
                yield from gen

        # explicit program-order interleave: attention for group g alternates
        # with the k/v projections of group g (needed only at the diagonal)
        # and the q projection of group g+1, so the PE stays fed through the
        # exp-bound attention phases
        drain(gen_proj_q(0))
        drain(gen_proj_kv(0))
        for sg in range(NG):
            a = gen_attn(sg)
            fs = []
            if sg > 0:
                fs.append(gen_proj_kv(sg))
            if sg + 1 < NG:
                fs.append(gen_proj_q(sg + 1))
            f = chain(*fs)
            while True:
                sf = next(f, StopIteration)
                sa = next(a, StopIteration)
                if sa is StopIteration and sf is StopIteration:
                    break


def build():
    nc = bacc.Bacc(
        "TRN2", target_bir_lowering=False, debug=False, num_devices=NCORES
    )
    x = nc.dram_tensor("x", [D, S], bf16, kind="ExternalInput")
    wq = nc.dram_tensor("wq", [D, HD], bf16, kind="ExternalInput")
    wk = nc.dram_tensor("wk", [D, HD], bf16, kind="ExternalInput")
    wv = nc.dram_tensor("wv", [D, HD], bf16, kind="ExternalInput")
    bq = nc.dram_tensor("bq", [128, 2], f32, kind="ExternalInput")
    bk = nc.dram_tensor("bk", [128, 2], f32, kind="ExternalInput")
    bvr = nc.dram_tensor("bvr", [128, HD], bf16, kind="ExternalInput")
    tril = nc.dram_tensor("tril", [128, 128], bf16, kind="ExternalInput")
    amask = nc.dram_tensor("amask", [128, 128], bf16, kind="ExternalInput")
    out = nc.dram_tensor("out", [NHC * 65, S], bf16, kind="ExternalOutput")
    with tile.TileContext(nc) as tc:
        _body(
            nc, tc, x.ap(), wq.ap(), wk.ap(), wv.ap(),
            bq.ap(), bk.ap(), bvr.ap(), tril.ap(), amask.ap(), out.ap(),
        )
    nc.compile()
    return nc


_NC_CACHE = None


def _get_nc():
    global _NC_CACHE
    if _NC_CACHE is None:
        _NC_CACHE = build()
    return _NC_CACHE


def make_in_maps(q_input, W_q, b_q, W_k, b_k, W_v, b_v):
    bfl = ml_dtypes.bfloat16
    ii = np.arange(128)
    tril = (ii[None, :] >= ii[:, None]).astype(bfl)
    amask = np.where(ii[None, :] >= ii[:, None], 0.0, -8.0e9).astype(bfl)
    xT = {
        b: np.ascontiguousarray(
            np.asarray(q_input[b], dtype=np.float32).T
        ).astype(bfl)
        for b in range(B)
    }
    in_maps = []
    for c in range(NCORES):
        b = c // 4
        hs = slice((c % 4) * HD, (c % 4 + 1) * HD)
        in_maps.append(
            {
                "x": xT[b],
                "wq": np.ascontiguousarray(
                    np.asarray(W_q[hs], dtype=np.float32).T
                ).astype(bfl),
                "wk": np.ascontiguousarray(
                    np.asarray(W_k[hs], dtype=np.float32).T
                ).astype(bfl),
                "wv": np.ascontiguousarray(
                    np.asarray(W_v[hs], dtype=np.float32).T
                ).astype(bfl),
                "bq": np.ascontiguousarray(
                    np.asarray(b_q[hs], dtype=np.float32).reshape(2, 128).T
                ),
                "bk": np.ascontiguousarray(
                    np.asarray(b_k[hs], dtype=np.float32).reshape(2, 128).T
                ),
                "bvr": np.ascontiguousarray(
                    np.broadcast_to(
                        np.asarray(b_v[hs], dtype=np.float32)[None, :],
                        (128, HD),
                    )
                ).astype(bfl),
                "tril": tril,
                "amask": amask,
            }
        )
    return in_maps


def assemble(results):
    full = np.empty((B, S, D), dtype=np.float32)
    for c in range(NCORES):
        b = c // 4
        h0 = (c % 4) * HD
        arr = results[c]["out"].astype(np.float32)  # [4*65, S]
        for h in range(NHC):
            zh = arr[65 * h : 65 * h + 64] / arr[65 * h + 64 : 65 * h + 65]
            full[b, :, h0 + 64 * h : h0 + 64 * (h + 1)] = zh.T
    return full


def _ensure_ntff_hook():
    """Register the axon NTFF profiling hook if the image's antenv lacks it."""
    try:
        from antenv import axon_hooks  # noqa: F401

        return
    except ImportError:
        pass
    import types

    try:
        from trn_agent_boot.trn_boot import _ntff_profile_via_ctypes

        hook = _ntff_profile_via_ctypes("/opt/axon/libaxon_pjrt.so")
    except Exception:
        hook = None
    mod = types.ModuleType("antenv.axon_hooks")
    mod._hook = hook
    mod.get_axon_ntff_profile_hook = lambda: mod._hook

    def _set(h):
        mod._hook = h

    mod.set_axon_ntff_profile_hook = _set
    sys.modules["antenv.axon_hooks"] = mod
    try:
        import antenv

        antenv.axon_hooks = mod
    except ImportError:
        pass


def run(inputs_dict, trace=False):
    """Run on hardware; returns (full_output, BassKernelResults)."""
    nc = _get_nc()
    if trace:
        _ensure_ntff_hook()
        import concourse.bass_utils as _bu

        _bu.upload_artifacts = lambda d: d  # no bucket access in this env
    in_maps = make_in_maps(**{k: np.asarray(v) for k, v in inputs_dict.items()})
    res = run_bass_kernel_spmd(nc, in_maps, core_ids=list(range(NCORES)), trace=trace)
    return assemble(res.results), res


def kernel(**inputs):
    out, _ = run(inputs, trace=False)
    return out


# revision 15
# speedup vs baseline: 1.1395x; 1.1395x over previous
"""Causal multi-head self-attention on 8 TRN2 NeuronCores.

Sharding: batch (2) x head-group (4 heads = 256 contiguous features) -> 8 cores.
Each core computes q/k/v projections for its 256 output features from its
batch's full activations, then causal attention for its 4 heads. No
collectives: the host concatenates the 8 shards.

Kernel layout choices (v3, bf16):
  - the host pre-transposes and casts x -> xT [D, S] bf16 and W -> W^T
    [D, 256] bf16, so the device does NO transposes and no PSUM->SBUF
    staging copies for x or W; all matmuls run bf16 (1 cyc/col on the PE,
    vs ~2 cyc/col measured for fp32r at K=64/M=65).
  - qT/kT stored [d, s] (head dim on partitions) so scores come out
    [k, q]; PV consumes exp(scores) directly as the moving operand.
  - score matmuls for a head pair (partitions 0-63 / 64-127) are emitted
    back-to-back: the K=64 stationaries land in disjoint PE row groups
    (tile_position auto-derived from base_partition), so the two matmuls
    execute concurrently in the array -> ~2x on the score stage.
  - softmax skips max-subtraction (scores ~ N(0,1) after the 1/8 scale).
  - causal mask: multiply exp by a 0/1 triangular mask on the diagonal
    128x128 blocks (post-exp, bf16, on DVE);
    fully-masked column windows are never computed.
  - scores for two consecutive key blocks share one 2-bank PSUM tile and
    (off-diagonal) one [128,1024] exp instruction, halving ACT overhead.
  - row sums ride along PV via a ones-column appended to V (65-wide PV
    stationary). Normalization transposes PV output back to [q, hd] on the
    PE (per-partition reciprocals are ~30x cheaper on DVE than per-column
    ones); the PSUM->SBUF staging copy of PV output rides on the idle DMA
    engines instead of DVE.
  - b_v is added during the v_aug staging copy against a host-replicated
    [128, 256] bias tile (no bias matmuls); b_q/b_k ride in the qT/kT
    staging tensor_scalar ops as per-partition scalars.
  - the schedule is emitted as generators explicitly interleaved in
    program order: attention for query group g interleaves with the
    projection units of s-group g+1 (causality makes group g
    data-complete after s-group g).
"""

import sys

import numpy as np

sys.path.insert(0, "/opt/trn_rl_repo")

import ml_dtypes

import concourse.bass as bass
import concourse.tile as tile
from concourse import bacc, mybir
from concourse.bass_utils import run_bass_kernel_spmd

B, S, D, H, DK = 2, 2048, 1024, 16, 64
NCORES = 8
HD = 256  # output features per core (4 heads x 64)
NHC = 4  # heads per core
NST = S // 128  # 16 s-tiles
NCC = D // 128  # 8 contraction chunks
NG = S // 512  # 4 query groups of 512

f32 = mybir.dt.float32
bf16 = mybir.dt.bfloat16
AF = mybir.ActivationFunctionType
PSUM = bass.MemorySpace.PSUM


def _body(nc, tc, x, wq, wk, wv, bq, bk, bvr, tril, out):
    with (
        tc.tile_pool(name="persist", bufs=1) as persist,
        tc.tile_pool(name="u", bufs=8) as u_pool,
        tc.tile_pool(name="zc", bufs=6) as zc_pool,
        tc.tile_pool(name="small", bufs=4) as small,
        tc.tile_pool(name="psum_sp", bufs=1, space=PSUM) as psum_sp,
        tc.tile_pool(name="psum_zp", bufs=1, space=PSUM) as psum_zp,
        tc.tile_pool(name="psum_pp", bufs=2, space=PSUM) as psum_pp,
    ):
        # ---- constants / small inputs ----
        tril_sb = persist.tile([128, 128], bf16)
        nc.sync.dma_start(out=tril_sb[:], in_=tril)
        bq_sb = persist.tile([128, 2], f32)
        bk_sb = persist.tile([128, 2], f32)
        bvr_sb = persist.tile([128, HD], bf16)
        nc.sync.dma_start(out=bq_sb[:], in_=bq)
        nc.sync.dma_start(out=bk_sb[:], in_=bk)
        nc.sync.dma_start(out=bvr_sb[:], in_=bvr)

        # preload the exp activation table while the x DMAs run
        warm = small.tile([1, 4], f32, tag="warm", name="warm")
        nc.scalar.activation(warm[:], tril_sb[0:1, 0:4], AF.Exp)

        # ---- weights (pre-transposed on host): [128, cc, hd] ----
        wq_sb = persist.tile([128, NCC, HD], bf16)
        wk_sb = persist.tile([128, NCC, HD], bf16)
        wv_sb = persist.tile([128, NCC, HD], bf16)
        for w_ext, w_sb in ((wq, wq_sb), (wk, wk_sb), (wv, wv_sb)):
            nc.sync.dma_start(
                out=w_sb[:], in_=w_ext.rearrange("(c p) d -> p c d", p=128)
            )

        # ---- x (pre-transposed on host): per-group tiles [128, cc, 512] ----
        xg = []
        for g in range(NG):
            xt = persist.tile([128, NCC, 512], bf16, name=f"xg{g}")
            nc.sync.dma_start(
                out=xt[:],
                in_=x.rearrange("(c p) s -> p c s", p=128)[
                    :, :, bass.ts(g, 512)
                ],
            )
            xg.append(xt)

        qT = persist.tile([128, 2, S], bf16)
        kT = persist.tile([128, 2, S], bf16)
        v_aug = persist.tile([128, NST, NHC, 65], bf16)

        # ones column of v_aug
        nc.vector.memset(v_aug[:, :, :, 64], 1.0)

        # ---- projections for s-group sg (512 queries = 4 s-tiles) ----
        # split into q-only and k/v parts: attention group g needs qT(g) for
        # all its score matmuls but kT(g)/v(g) only at the diagonal blocks,
        # so the k/v projections of group g overlap attention of group g
        def proj_one(w_sb, bias, dstT, sg):
            xt = xg[sg]
            pa = psum_pp.tile([128, 512], f32, tag="pp", name="pa")
            pb = psum_pp.tile([128, 512], f32, tag="pp", name="pb")
            for cc in range(NCC):
                for hdc, pp in ((0, pa), (1, pb)):
                    nc.tensor.matmul(
                        pp[:],
                        lhsT=w_sb[:, cc, bass.ts(hdc, 128)],
                        rhs=xt[:, cc, :],
                        start=(cc == 0),
                        stop=(cc == NCC - 1),
                    )
            for hdc, pp in ((0, pa), (1, pb)):
                nc.vector.tensor_scalar_add(
                    dstT[:, hdc, bass.ts(sg, 512)],
                    pp[:],
                    bias[:, hdc : hdc + 1],
                )

        def gen_proj_q(sg):
            proj_one(wq_sb, bq_sb, qT, sg)
            yield

        def gen_proj_kv(sg):
            proj_one(wk_sb, bk_sb, kT, sg)
            yield
            # v projection for the 4 s-tiles (pairs, alternating banks);
            # b_v is added during the staging copy against the replicated
            # bias tile
            xt = xg[sg]
            for spair in range(2):
                pvs = [
                    psum_pp.tile([128, HD], f32, tag="pp", name=f"pv{stl}")
                    for stl in range(2)
                ]
                for cc in range(NCC):
                    for stl in range(2):
                        nc.tensor.matmul(
                            pvs[stl][:],
                            lhsT=xt[:, cc, bass.ts(spair * 2 + stl, 128)],
                            rhs=wv_sb[:, cc, :],
                            start=(cc == 0),
                            stop=(cc == NCC - 1),
                        )
                for stl in range(2):
                    st = sg * 4 + spair * 2 + stl
                    nc.vector.tensor_add(
                        v_aug[:, st, :, 0:64],
                        pvs[stl][:].rearrange("p (h d) -> p h d", h=NHC),
                        bvr_sb[:].rearrange("p (h d) -> p h d", h=NHC),
                    )
                yield

        # ---- attention for query group g (queries [512g, 512g+512)) ----
        def gen_attn(g):
            nkc = 4 * g + 4
            for pair in range(2):
                hdc = pair
                heads = (2 * pair, 2 * pair + 1)
                zp = {}
                for ci, h in enumerate(heads):
                    zp[h] = psum_zp.tile(
                        [65, 512], f32, tag=f"zp{ci}", name=f"zp{h}"
                    )
                prev = []

                def flush_pv(prev):
                    for pkb, h, u in prev:
                        for ci, kc in enumerate((pkb, pkb + 1)):
                            q0 = max(0, 128 * (kc - 4 * g))
                            nc.tensor.matmul(
                                zp[h][:, q0:512],
                                lhsT=v_aug[:, kc, h, :],
                                rhs=u[:, 512 * ci + q0 : 512 * (ci + 1)],
                                start=(kc == 0),
                                stop=(kc == nkc - 1),
                            )

                for kb in range(0, nkc, 2):
                    sp = {}
                    for ci, h in enumerate(heads):
                        sp[h] = psum_sp.tile(
                            [128, 1024], f32, tag=f"sp{ci}", name=f"sp{h}"
                        )
                    # 4 score MMs; head pair adjacent -> concurrent row tiles
                    for ci, kc in enumerate((kb, kb + 1)):
                        j = kc - 4 * g
                        q0 = max(0, 128 * j)
                        for h in heads:
                            po = (h % 2) * 64
                            nc.tensor.matmul(
                                sp[h][:, 512 * ci + q0 : 512 * (ci + 1)],
                                lhsT=kT[po : po + 64, hdc, bass.ts(kc, 128)],
                                rhs=qT[
                                    po : po + 64,
                                    hdc,
                                    bass.ds(g * 512 + q0, 512 - q0),
                                ],
                                start=True,
                                stop=True,
                            )
                    # exp (one [128,1024] instr off-diagonal, windowed on diag)
                    cur = []
                    diag = kb >= 4 * g
                    for h in heads:
                        u = u_pool.tile([128, 1024], bf16, tag="u", name="u")
                        if diag:
                            for ci, kc in enumerate((kb, kb + 1)):
                                q0 = 128 * (kc - 4 * g)
                                nc.scalar.activation(
                                    u[:, 512 * ci + q0 : 512 * (ci + 1)],
                                    sp[h][:, 512 * ci + q0 : 512 * (ci + 1)],
                                    AF.Exp,
                                    scale=0.125,
                                )
                        else:
                            nc.scalar.activation(
                                u[:], sp[h][:], AF.Exp, scale=0.125
                            )
                        cur.append((h, u))
                    if diag:
                        for h, u in cur:
                            for ci, kc in enumerate((kb, kb + 1)):
                                qd = 512 * ci + 128 * (kc - 4 * g)
                                nc.vector.tensor_mul(
                                    u[:, qd : qd + 128],
                                    u[:, qd : qd + 128],
                                    tril_sb[:],
                                )
                    # PV for the previous kb's exp tiles (software pipeline)
                    flush_pv(prev)
                    prev = [(kb, h, u) for h, u in cur]
                    yield
                flush_pv(prev)
                # stage unnormalized PV output (incl. the sums row) to SBUF
                # and ship it; the host divides by the sums row during
                # unsharding
                for h in heads:
                    zc = zc_pool.tile([65, 512], bf16, tag="zc", name="zc")
                    nc.vector.tensor_copy(zc[:], zp[h][:])
                    nc.sync.dma_start(
                        out=out[bass.ds(65 * h, 65), bass.ts(g, 512)],
                        in_=zc[:],
                    )
                    yield

        def drain(gen):
            for _ in gen:
                pass

        def chain(*gens):
            for gen in gens:
                yield from gen

        # explicit program-order interleave: attention for group g alternates
        # with the k/v projections of group g (needed only at the diagonal)
        # and the q projection of group g+1, so the PE stays fed through the
        # exp-bound attention phases
        drain(gen_proj_q(0))
        drain(gen_proj_kv(0))
        for sg in range(NG):
            a = gen_attn(sg)
            fs = []
            if sg > 0:
                fs.append(gen_proj_kv(sg))
            if sg + 1 < NG:
                fs.append(gen_proj_q(sg + 1))
            f = chain(*fs)
            while True:
                sf = next(f, StopIteration)
                sa = next(a, StopIteration)
                if sa is StopIteration and sf is StopIteration:
                    break


def build():
    nc = bacc.Bacc(
        "TRN2", target_bir_lowering=False, debug=False, num_devices=NCORES
    )
    x = nc.dram_tensor("x", [D, S], bf16, kind="ExternalInput")
    wq = nc.dram_tensor("wq", [D, HD], bf16, kind="ExternalInput")
    wk = nc.dram_tensor("wk", [D, HD], bf16, kind="ExternalInput")
    wv = nc.dram_tensor("wv", [D, HD], bf16, kind="ExternalInput")
    bq = nc.dram_tensor("bq", [128, 2], f32, kind="ExternalInput")
    bk = nc.dram_tensor("bk", [128, 2], f32, kind="ExternalInput")
    bvr = nc.dram_tensor("bvr", [128, HD], bf16, kind="ExternalInput")
    tril = nc.dram_tensor("tril", [128, 128], bf16, kind="ExternalInput")
    out = nc.dram_tensor("out", [NHC * 65, S], bf16, kind="ExternalOutput")
    with tile.TileContext(nc) as tc:
        _body(
            nc, tc, x.ap(), wq.ap(), wk.ap(), wv.ap(),
            bq.ap(), bk.ap(), bvr.ap(), tril.ap(), out.ap(),
        )
    nc.compile()
    return nc


_NC_CACHE = None


def _get_nc():
    global _NC_CACHE
    if _NC_CACHE is None:
        _NC_CACHE = build()
    return _NC_CACHE


def make_in_maps(q_input, W_q, b_q, W_k, b_k, W_v, b_v):
    bfl = ml_dtypes.bfloat16
    ii = np.arange(128)
    tril = (ii[None, :] >= ii[:, None]).astype(bfl)
    xT = {
        b: np.ascontiguousarray(
            np.asarray(q_input[b], dtype=np.float32).T
        ).astype(bfl)
        for b in range(B)
    }
    in_maps = []
    for c in range(NCORES):
        b = c // 4
        hs = slice((c % 4) * HD, (c % 4 + 1) * HD)
        in_maps.append(
            {
                "x": xT[b],
                "wq": np.ascontiguousarray(
                    np.asarray(W_q[hs], dtype=np.float32).T
                ).astype(bfl),
                "wk": np.ascontiguousarray(
                    np.asarray(W_k[hs], dtype=np.float32).T
                ).astype(bfl),
                "wv": np.ascontiguousarray(
                    np.asarray(W_v[hs], dtype=np.float32).T
                ).astype(bfl),
                "bq": np.ascontiguousarray(
                    np.asarray(b_q[hs], dtype=np.float32).reshape(2, 128).T
                ),
                "bk": np.ascontiguousarray(
                    np.asarray(b_k[hs], dtype=np.float32).reshape(2, 128).T
                ),
                "bvr": np.ascontiguousarray(
                    np.broadcast_to(
                        np.asarray(b_v[hs], dtype=np.float32)[None, :],
                        (128, HD),
                    )
                ).astype(bfl),
                "tril": tril,
            }
        )
    return in_maps


def assemble(results):
    full = np.empty((B, S, D), dtype=np.float32)
    for c in range(NCORES):
        b = c // 4
        h0 = (c % 4) * HD
        arr = results[c]["out"].astype(np.float32)  # [4*65, S]
        for h in range(NHC):
            zh = arr[65 * h : 65 * h + 64] / arr[65 * h + 64 : 65 * h + 65]
            full[b, :, h0 + 64 * h : h0 + 64 * (h + 1)] = zh.T
    return full


def _ensure_ntff_hook():
    """Register the axon NTFF profiling hook if the image's antenv lacks it."""
    try:
        from antenv import axon_hooks  # noqa: F401

        return
    except ImportError:
        pass
    import types

    try:
        from trn_agent_boot.trn_boot import _ntff_profile_via_ctypes

        hook = _ntff_profile_via_ctypes("/opt/axon/libaxon_pjrt.so")
    except Exception:
        hook = None
    mod = types.ModuleType("antenv.axon_hooks")
    mod._hook = hook
    mod.get_axon_ntff_profile_hook = lambda: mod._hook

    def _set(h):
        mod._hook = h

    mod.set_axon_ntff_profile_hook = _set
    sys.modules["antenv.axon_hooks"] = mod
    try:
        import antenv

        antenv.axon_hooks = mod
    except ImportError:
        pass


def run(inputs_dict, trace=False):
    """Run on hardware; returns (full_output, BassKernelResults)."""
    nc = _get_nc()
    if trace:
        _ensure_ntff_hook()
        import concourse.bass_utils as _bu

        _bu.upload_artifacts = lambda d: d  # no bucket access in this env
    in_maps = make_in_maps(**{k: np.asarray(v) for k, v in inputs_dict.items()})
    res = run_bass_kernel_spmd(nc, in_maps, core_ids=list(range(NCORES)), trace=trace)
    return assemble(res.results), res


def kernel(**inputs):
    out, _ = run(inputs, trace=False)
    return out
